# revision 17
# baseline (speedup 1.0000x reference)
"""Trainium2 Bass kernel for nn_Add_PairLinears.

y = sum_a( blockdiag2(W[a]) applied to x[:, perms[a]] ) + sum_a b[a]

Strategy (data-parallel over batch, 8 cores, no collectives):
  - Each core owns a batch shard of 1024 rows.
  - On device: transpose x shard to x^T (d on partitions) in bf16 via PE,
    store x^T to DRAM, then for each mixer a gather the permuted rows with
    SWDGE dma_gather (perm values baked into int16 index tables).
  - The 2x2 block-diagonal mix + sum over the 8 mixers is 8 accumulating
    128x128 bf16 matmuls per output d-tile into PSUM (the pair weights are
    expanded on host into block-diagonal 128x128 lhsT tiles).
  - PSUM is evacuated through the scalar engine with the per-partition
    bias sum_a b[a] fused in; output is stored transposed (y^T) and
    unsharded/transposed back on host.
"""

import numpy as np
import ml_dtypes

import concourse.bass as bass
import concourse.bacc as bacc
import concourse.tile as tile
from concourse import library_config, mybir
from concourse.bass_utils import run_bass_kernel_spmd

B, D, A = 8192, 4096, 8
N_CORES = 8
BC = B // N_CORES          # 1024 batch rows per core
NJ = D // 128              # 32 d-tiles of 128
JG = 4                     # j-tiles per gather group
NG = NJ // JG              # 8 gather groups per mixer

F32 = mybir.dt.float32
BF16 = mybir.dt.bfloat16
I16 = mybir.dt.int16

_GRAPH_CACHE = {}
_LAST_RESULTS = None

import os
HOST_XT = os.environ.get("HOST_XT", "0") == "1"   # feed x^T bf16 from host
SKIP_MIX = os.environ.get("SKIP_MIX", "0") == "1"  # bisect: skip gather+matmul


def _build_graph():
    nc = bacc.Bacc(None)

    if HOST_XT:
        xt_ext = nc.declare_dram_parameter("xt", [D, BC], BF16, isOutput=False)
    else:
        x_ext = nc.declare_dram_parameter("x", [BC, D], F32, isOutput=False)
    lhsT_ext = nc.declare_dram_parameter("lhsT", [NJ, 128, A * 128], BF16, isOutput=False)
    idx_ext = nc.declare_dram_parameter("idx", [128, A * 256], I16, isOutput=False)
    bsum_ext = nc.declare_dram_parameter("bsum", [128, NJ], F32, isOutput=False)
    ident_ext = nc.declare_dram_parameter("ident", [128, 128], F32, isOutput=False)
    yt_ext = nc.declare_dram_parameter("yt", [D, BC], F32, isOutput=True)

    with tile.TileContext(nc) as tc:
        with (
            tc.tile_pool(name="const", bufs=1) as constp,
            tc.tile_pool(name="xin", bufs=2) as xinp,
            tc.tile_pool(name="xt", bufs=1) as xtp,
            tc.tile_pool(name="lhs", bufs=2) as lhsp,
            tc.tile_pool(name="g", bufs=8) as gp,
            tc.tile_pool(name="y", bufs=2) as yp,
            tc.tile_pool(name="pst", bufs=4, space="PSUM") as pstp,
            tc.tile_pool(name="psm", bufs=4, space="PSUM") as psmp,
            tc.tile_pool(name="dram", bufs=1, space="DRAM") as dramp,
        ):
            nc.gpsimd.load_library(library_config.mlp)

            ident = constp.tile([128, 128], F32)
            nc.sync.dma_start(out=ident[:], in_=ident_ext[:])
            idx_sb = constp.tile([128, A * 256], I16)
            nc.sync.dma_start(out=idx_sb[:], in_=idx_ext[:])
            bsum_sb = constp.tile([128, NJ], F32)
            nc.sync.dma_start(out=bsum_sb[:], in_=bsum_ext[:])

            # x^T resident in SBUF: [partition = d%128, (d//128, batch)]
            xt_sb = xtp.tile([128, NJ, BC], BF16)
            yt_v = yt_ext[:].rearrange("(j p) b -> p j b", p=128)

            if HOST_XT:
                xt_dram = xt_ext
                xt_v = xt_ext[:].rearrange("(j p) b -> p j b", p=128)
                for j in range(NJ):
                    nc.sync.dma_start(out=xt_sb[:, j, :], in_=xt_v[:, j, :])
            else:
                # x^T copy in DRAM for the row gathers: row d, col batch
                xt_dram = dramp.tile([D, BC], BF16)
                xt_dram_v = xt_dram[:].rearrange("(j p) b -> p j b", p=128)

                # ---- Phase 1: load + transpose + cast ----
                JCH = 8  # j-tiles per x-load chunk
                for bt in range(BC // 128):
                    for jg0 in range(0, NJ, JCH):
                        xtile = xinp.tile([128, JCH * 128], F32)
                        nc.sync.dma_start(
                            out=xtile[:],
                            in_=x_ext[bt * 128:(bt + 1) * 128,
                                      jg0 * 128:(jg0 + JCH) * 128])
                        for jo in range(JCH):
                            j = jg0 + jo
                            pt = pstp.tile([128, 128], F32)
                            nc.tensor.transpose(pt[:], xtile[:, jo * 128:(jo + 1) * 128], ident[:])
                            nc.vector.tensor_copy(xt_sb[:, j, bt * 128:(bt + 1) * 128], pt[:])

                # ---- Phase 2: spill x^T (bf16) to DRAM for the gathers ----
                for j in range(NJ):
                    nc.sync.dma_start(out=xt_dram_v[:, j, :], in_=xt_sb[:, j, :])

            if SKIP_MIX:
                for j in range(NJ):
                    ytile = yp.tile([128, BC], F32)
                    nc.vector.tensor_copy(ytile[:], xt_sb[:, j, :])
                    nc.sync.dma_start(out=yt_v[:, j, :], in_=ytile[:])

            # ---- Phase 3: gather + mix + store ----
            for gi in range(NG if not SKIP_MIX else 0):
                gts = {}
                for a in range(1, A):
                    gt = gp.tile([128, JG, BC], BF16, tag="g")
                    c0 = a * 256 + gi * 32
                    nc.gpsimd.dma_gather(
                        out_ap=gt[:],
                        in_ap=xt_dram[:],
                        idxs_ap=idx_sb[:, c0:c0 + 32],
                        num_idxs=JG * 128,
                        num_idxs_reg=JG * 128,
                        elem_size=BC,
                    )
                    gts[a] = gt
                for jc in range(JG):
                    j = gi * JG + jc
                    lhs_sb = lhsp.tile([128, A * 128], BF16)
                    nc.sync.dma_start(out=lhs_sb[:], in_=lhsT_ext[j])
                    ytile = yp.tile([128, BC], F32)
                    for ch in range(2):
                        pm = psmp.tile([128, 512], F32)
                        for a in range(A):
                            if a == 0:
                                rhs = xt_sb[:, j, ch * 512:(ch + 1) * 512]
                            else:
                                rhs = gts[a][:, jc, ch * 512:(ch + 1) * 512]
                            nc.tensor.matmul(
                                pm[:],
                                lhs_sb[:, a * 128:(a + 1) * 128],
                                rhs,
                                start=(a == 0),
                                stop=(a == A - 1),
                            )
                        nc.scalar.activation(
                            ytile[:, ch * 512:(ch + 1) * 512],
                            pm[:],
                            mybir.ActivationFunctionType.Identity,
                            bias=bsum_sb[:, j:j + 1],
                        )
                    nc.sync.dma_start(out=yt_v[:, j, :], in_=ytile[:])

    nc.compile()
    return nc


def _host_tables(W, b, perms):
    """Build the device-side constant tables from W/b/perms."""
    # lhsT[j, t, a, o]: weight applied to gathered row t (= x^T[perms[a, 128j+t]])
    # contributing to output row 128j+o.  Output 2n+oo uses inputs
    # perms[a, 2n+i] with weight W[a, n, i, oo]; within tile j, t = 2m+i,
    # o = 2m+oo for pair m = n - 64j.
    Wr = W.reshape(A, NJ, 64, 2, 2)
    lhsT = np.zeros((NJ, 128, A, 128), np.float32)
    m = np.arange(64)
    for i in range(2):
        for oo in range(2):
            # paired advanced indexing on axes 1 and 3 -> result axes [64, NJ, A]
            lhsT[:, 2 * m + i, :, 2 * m + oo] = Wr[:, :, :, i, oo].transpose(2, 1, 0)
    lhsT = np.ascontiguousarray(lhsT.reshape(NJ, 128, A * 128)).astype(ml_dtypes.bfloat16)

    # idx: per mixer, perm values wrapped over 16 partitions (index i at
    # [i%16, i//16]), replicated into each Q7 core's 16-partition group
    idx = np.zeros((128, A * 256), np.int16)
    for a in range(A):
        w16 = perms[a].astype(np.int16).reshape(256, 16).T
        idx[:, a * 256:(a + 1) * 256] = np.tile(w16, (8, 1))

    bsum = np.ascontiguousarray(b.astype(np.float64).sum(axis=0).astype(np.float32).reshape(NJ, 128).T)
    ident = np.eye(128, dtype=np.float32)
    return lhsT, idx, bsum, ident


def kernel(x, W, b, perms):
    x = np.asarray(x, dtype=np.float32)
    W = np.asarray(W, dtype=np.float32)
    b = np.asarray(b, dtype=np.float32)
    perms = np.asarray(perms)

    lhsT, idx, bsum, ident = _host_tables(W, b, perms)

    if "nc" not in _GRAPH_CACHE:
        _GRAPH_CACHE["nc"] = _build_graph()
    nc = _GRAPH_CACHE["nc"]

    in_maps = []
    for c in range(N_CORES):
        m = {
            "lhsT": lhsT,
            "idx": idx,
            "bsum": bsum,
            "ident": ident,
        }
        xs = x[c * BC:(c + 1) * BC]
        if HOST_XT:
            m["xt"] = np.ascontiguousarray(xs.T).astype(ml_dtypes.bfloat16)
        else:
            m["x"] = np.ascontiguousarray(xs)
        in_maps.append(m)

    res = run_bass_kernel_spmd(nc, in_maps, core_ids=list(range(N_CORES)))
    global _LAST_RESULTS
    _LAST_RESULTS = res
    y = np.concatenate(
        [np.asarray(res.results[c]["yt"], dtype=np.float32).T for c in range(N_CORES)],
        axis=0,
    )
    return np.ascontiguousarray(y)


# revision 19
# speedup vs baseline: 1.3549x; 1.3549x over previous
"""Trainium2 Bass kernel for nn_Add_PairLinears.

y = sum_a( blockdiag2(W[a]) applied to x[:, perms[a]] ) + sum_a b[a]

Strategy (data-parallel over batch, 8 cores, no collectives):
  - Each core owns a batch shard of 1024 rows.
  - On device: cast x to bf16, transpose to x^T (d on partitions) via PE,
    spill x^T to DRAM, then for each mixer a>0 gather the permuted rows
    with SWDGE dma_gather (perm values baked into int16 index tables,
    4 SWDGE queues round-robin). Mixer 0 reads the SBUF-resident x^T.
  - The 2x2 block-diagonal mix + sum over the 8 mixers is 8 accumulating
    128x128 bf16 matmuls per output d-tile into PSUM (pair weights are
    expanded on host into block-diagonal 128x128 lhsT tiles).
  - PSUM is evacuated through the scalar engine with the per-partition
    bias sum_a b[a] fused in; output is stored transposed (y^T, bf16) and
    unsharded/transposed/upcast on host.
"""

import os

import numpy as np
import ml_dtypes

import concourse.bass as bass
import concourse.bacc as bacc
import concourse.tile as tile
from concourse import library_config, mybir
from concourse.bass_utils import run_bass_kernel_spmd

B, D, A = 8192, 4096, 8
N_CORES = 8
BC = B // N_CORES          # 1024 batch rows per core
NJ = D // 128              # 32 d-tiles of 128
JG = 4                     # j-tiles per gather group
NG = NJ // JG              # 8 gather groups per mixer
NQ = 4                     # SWDGE queues

F32 = mybir.dt.float32
BF16 = mybir.dt.bfloat16
I16 = mybir.dt.int16

_GRAPH_CACHE = {}
_LAST_RESULTS = None

HOST_XT = os.environ.get("HOST_XT", "0") == "1"   # feed x^T bf16 from host


def _build_graph():
    nc = bacc.Bacc(None, num_swdge_queues=NQ)

    if HOST_XT:
        xt_ext = nc.declare_dram_parameter("xt", [D, BC], BF16, isOutput=False)
    else:
        x_ext = nc.declare_dram_parameter("x", [BC, D], F32, isOutput=False)
    lhsT_ext = nc.declare_dram_parameter("lhsT", [NJ, 128, A * 128], BF16, isOutput=False)
    idx_ext = nc.declare_dram_parameter("idx", [128, A * 256], I16, isOutput=False)
    bsum_ext = nc.declare_dram_parameter("bsum", [128, NJ], F32, isOutput=False)
    ident_ext = nc.declare_dram_parameter("ident", [128, 128], BF16, isOutput=False)
    yt_ext = nc.declare_dram_parameter("yt", [D, BC], BF16, isOutput=True)

    qn = [0]

    def next_q():
        q = qn[0]
        qn[0] = (q + 1) % NQ
        return q

    with tile.TileContext(nc) as tc:
        with (
            tc.tile_pool(name="const", bufs=1) as constp,
            tc.tile_pool(name="xin", bufs=3) as xinp,
            tc.tile_pool(name="xbf", bufs=3) as xbfp,
            tc.tile_pool(name="xt", bufs=1) as xtp,
            tc.tile_pool(name="lhs", bufs=3) as lhsp,
            tc.tile_pool(name="g", bufs=8) as gp,
            tc.tile_pool(name="y", bufs=3) as yp,
            tc.tile_pool(name="pst", bufs=4, space="PSUM") as pstp,
            tc.tile_pool(name="psm", bufs=4, space="PSUM") as psmp,
            tc.tile_pool(name="dram", bufs=1, space="DRAM") as dramp,
        ):
            nc.gpsimd.load_library(library_config.mlp)

            ident = constp.tile([128, 128], BF16)
            nc.sync.dma_start(out=ident[:], in_=ident_ext[:])
            idx_sb = constp.tile([128, A * 256], I16)
            nc.sync.dma_start(out=idx_sb[:], in_=idx_ext[:])
            bsum_sb = constp.tile([128, NJ], F32)
            nc.sync.dma_start(out=bsum_sb[:], in_=bsum_ext[:])

            # x^T resident in SBUF: [partition = d%128, (d//128, batch)]
            xt_sb = xtp.tile([128, NJ, BC], BF16)
            yt_v = yt_ext[:].rearrange("(j p) b -> p j b", p=128)

            if HOST_XT:
                xt_dram = xt_ext
                xt_v = xt_ext[:].rearrange("(j p) b -> p j b", p=128)
                for j in range(NJ):
                    nc.sync.dma_start(out=xt_sb[:, j, :], in_=xt_v[:, j, :])
            else:
                # x^T copy in DRAM for the row gathers: row d, col batch
                xt_dram = dramp.tile([D, BC], BF16)
                xt_dram_v = xt_dram[:].rearrange("(j p) b -> p j b", p=128)

                # ---- Phase 1: load + cast bf16 + PE transpose ----
                JCH = 8  # j-tiles per x-load chunk
                for bt in range(BC // 128):
                    for jg0 in range(0, NJ, JCH):
                        xtile = xinp.tile([128, JCH * 128], F32)
                        nc.sync.dma_start(
                            out=xtile[:],
                            in_=x_ext[bt * 128:(bt + 1) * 128,
                                      jg0 * 128:(jg0 + JCH) * 128])
                        xb = xbfp.tile([128, JCH * 128], BF16)
                        nc.scalar.activation(
                            xb[:], xtile[:], mybir.ActivationFunctionType.Copy)
                        for jo in range(JCH):
                            j = jg0 + jo
                            pt = pstp.tile([128, 128], BF16)
                            nc.tensor.transpose(
                                pt[:], xb[:, jo * 128:(jo + 1) * 128], ident[:])
                            nc.vector.tensor_copy(
                                xt_sb[:, j, bt * 128:(bt + 1) * 128], pt[:])

                # ---- Phase 2: spill x^T (bf16) to DRAM for the gathers ----
                for j in range(NJ):
                    nc.sync.dma_start(out=xt_dram_v[:, j, :], in_=xt_sb[:, j, :])

            # ---- Phase 3: gather + mix + store ----
            for gi in range(NG):
                gts = {}
                for a in range(1, A):
                    gt = gp.tile([128, JG, BC], BF16, tag="g")
                    c0 = a * 256 + gi * 32
                    nc.gpsimd.dma_gather(
                        out_ap=gt[:],
                        in_ap=xt_dram[:],
                        idxs_ap=idx_sb[:, c0:c0 + 32],
                        num_idxs=JG * 128,
                        num_idxs_reg=JG * 128,
                        elem_size=BC,
                        queue_num=next_q(),
                    )
                    gts[a] = gt
                for jc in range(JG):
                    j = gi * JG + jc
                    lhs_sb = lhsp.tile([128, A * 128], BF16)
                    nc.sync.dma_start(out=lhs_sb[:], in_=lhsT_ext[j])
                    ytile = yp.tile([128, BC], BF16)
                    pm0 = psmp.tile([128, 512], F32, tag="pm")
                    pm1 = psmp.tile([128, 512], F32, tag="pm")
                    pms = [pm0, pm1]
                    for a in range(A):
                        rhs_full = xt_sb[:, j, :] if a == 0 else gts[a][:, jc, :]
                        for ch in range(2):
                            nc.tensor.matmul(
                                pms[ch][:],
                                lhs_sb[:, a * 128:(a + 1) * 128],
                                rhs_full[:, ch * 512:(ch + 1) * 512],
                                start=(a == 0),
                                stop=(a == A - 1),
                            )
                    for ch in range(2):
                        nc.scalar.activation(
                            ytile[:, ch * 512:(ch + 1) * 512],
                            pms[ch][:],
                            mybir.ActivationFunctionType.Identity,
                            bias=bsum_sb[:, j:j + 1],
                        )
                    nc.sync.dma_start(out=yt_v[:, j, :], in_=ytile[:])

    nc.compile()
    return nc


def _host_tables(W, b, perms):
    """Build the device-side constant tables from W/b/perms."""
    # lhsT[j, t, a, o]: weight applied to gathered row t (= x^T[perms[a, 128j+t]])
    # contributing to output row 128j+o.  Output 2n+oo uses inputs
    # perms[a, 2n+i] with weight W[a, n, i, oo]; within tile j, t = 2m+i,
    # o = 2m+oo for pair m = n - 64j.
    Wr = W.reshape(A, NJ, 64, 2, 2)
    lhsT = np.zeros((NJ, 128, A, 128), np.float32)
    m = np.arange(64)
    for i in range(2):
        for oo in range(2):
            # paired advanced indexing on axes 1 and 3 -> result axes [64, NJ, A]
            lhsT[:, 2 * m + i, :, 2 * m + oo] = Wr[:, :, :, i, oo].transpose(2, 1, 0)
    lhsT = np.ascontiguousarray(lhsT.reshape(NJ, 128, A * 128)).astype(ml_dtypes.bfloat16)

    # idx: per mixer, perm values wrapped over 16 partitions (index i at
    # [i%16, i//16]), replicated into each Q7 core's 16-partition group
    idx = np.zeros((128, A * 256), np.int16)
    for a in range(A):
        w16 = perms[a].astype(np.int16).reshape(256, 16).T
        idx[:, a * 256:(a + 1) * 256] = np.tile(w16, (8, 1))

    bsum = np.ascontiguousarray(
        b.astype(np.float64).sum(axis=0).astype(np.float32).reshape(NJ, 128).T)
    ident = np.eye(128, dtype=np.float32).astype(ml_dtypes.bfloat16)
    return lhsT, idx, bsum, ident


def kernel(x, W, b, perms):
    x = np.asarray(x, dtype=np.float32)
    W = np.asarray(W, dtype=np.float32)
    b = np.asarray(b, dtype=np.float32)
    perms = np.asarray(perms)

    lhsT, idx, bsum, ident = _host_tables(W, b, perms)

    if "nc" not in _GRAPH_CACHE:
        _GRAPH_CACHE["nc"] = _build_graph()
    nc = _GRAPH_CACHE["nc"]

    in_maps = []
    for c in range(N_CORES):
        m = {
            "lhsT": lhsT,
            "idx": idx,
            "bsum": bsum,
            "ident": ident,
        }
        xs = x[c * BC:(c + 1) * BC]
        if HOST_XT:
            m["xt"] = np.ascontiguousarray(xs.T).astype(ml_dtypes.bfloat16)
        else:
            m["x"] = np.ascontiguousarray(xs)
        in_maps.append(m)

    res = run_bass_kernel_spmd(nc, in_maps, core_ids=list(range(N_CORES)))
    global _LAST_RESULTS
    _LAST_RESULTS = res
    y = np.concatenate(
        [np.asarray(res.results[c]["yt"], dtype=np.float32).T for c in range(N_CORES)],
        axis=0,
    )
    return np.ascontiguousarray(y)
